# revision 31
# baseline (speedup 1.0000x reference)
"""Trainium2 Bass kernel: multi-head attention (B=2, T=2048, C=2048, H=16, D=128).

v2 reconstruction (A/B calibration vs v4 under current HW power state).

Sharding: tensor-parallel over heads. 8 cores x 2 heads each.
Per-core dataflow and scheduling: see kernel_v4.py docstring.
"""

import math

import numpy as np

N_CORES = 8
B, T, C = 2, 2048, 2048
N_HEAD, D = 16, 128
HPC = N_HEAD // N_CORES          # heads per core
JC = HPC * D                     # per-core slice width of qkv/proj dims

RT = 512                         # query tile (moving free dim) in attention
RP = 256                         # token tile for qkv projection
KB = 128                         # key block (contraction tile) in attention

PHASE_MARKS = []


def _build(Bp, Tp, Cp, hpc, d):
    PHASE_MARKS.clear()
    import concourse.bacc as bacc
    import concourse.tile as tile
    from concourse import mybir

    f32 = mybir.dt.float32
    bf16 = mybir.dt.bfloat16
    Exp = mybir.ActivationFunctionType.Exp
    Copy = mybir.ActivationFunctionType.Copy

    jc = hpc * d
    BT = Bp * Tp
    n_ck = Cp // 128
    n_rt = Tp // RP
    n_kb = Tp // KB
    n_qt = Tp // RT
    n_rb = Tp // 128
    n_ot = Cp // RT
    hd = d // 2
    scale = 1.0 / math.sqrt(d)

    nc = bacc.Bacc("TRN2", target_bir_lowering=False, debug=False)

    xT = nc.declare_dram_parameter("xT", [Cp, BT], bf16, isOutput=False)
    wqkv = nc.declare_dram_parameter("wqkv", [Cp, 3 * jc], bf16,
                                     isOutput=False)
    wp = nc.declare_dram_parameter("wp", [jc, Cp], bf16, isOutput=False)
    ones_d = nc.declare_dram_parameter("ones", [128, 1], f32, isOutput=False)
    cosT = nc.declare_dram_parameter("cosT", [d, Tp], f32, isOutput=False)
    sinT = nc.declare_dram_parameter("sinT", [d, Tp], f32, isOutput=False)
    out = nc.declare_dram_parameter("out", [BT, Cp], bf16, isOutput=True)

    with tile.TileContext(nc) as tc:
        with (
            nc.allow_low_precision(reason="bf16 matmuls, fp32 PSUM accum"),
            tc.tile_pool(name="wpool", bufs=1) as wpool,
            tc.tile_pool(name="acts", bufs=2) as acts,
            tc.tile_pool(name="xpool", bufs=2) as xpool,
            tc.tile_pool(name="rope", bufs=3) as rope,
            tc.tile_pool(name="epool", bufs=8) as epool,
            tc.tile_pool(name="dpool", bufs=4) as dpool,
            tc.tile_pool(name="small", bufs=3) as small,
            tc.tile_pool(name="opool", bufs=4) as opool,
            tc.tile_pool(name="pp", bufs=3, space="PSUM") as pp,
            tc.tile_pool(name="ps_s", bufs=2, space="PSUM") as ps_s,
            tc.tile_pool(name="ps_y", bufs=2, space="PSUM") as ps_y,
            tc.tile_pool(name="ps_fin", bufs=1, space="PSUM") as ps_fin,
        ):
            wq_sb, wk_sb, wv_sb = [], [], []
            for ck in range(n_ck):
                t = wpool.tile([128, 3 * jc], bf16, tag=f"w{ck}",
                               name=f"w{ck}")
                eng = nc.sync if ck % 2 == 0 else nc.scalar
                eng.dma_start(t, wqkv[ck * 128:(ck + 1) * 128, :])
                wq_sb.append(t[:, 0:jc])
                wk_sb.append(t[:, jc:2 * jc])
                wv_sb.append(t[:, 2 * jc:3 * jc])
            cos_sb = wpool.tile([d, Tp], f32, tag="cos")
            sin_sb = wpool.tile([d, Tp], f32, tag="sin")
            nc.scalar.dma_start(cos_sb, cosT[:])
            nc.scalar.dma_start(sin_sb, sinT[:])
            ones_sb = wpool.tile([128, 1], f32, tag="ones")
            nc.sync.dma_start(ones_sb, ones_d[:])
            ones_bf = wpool.tile([128, 1], bf16, tag="ones_bf")
            nc.vector.tensor_copy(out=ones_bf, in_=ones_sb)
            wp_sb = wpool.tile([128, hpc, Cp], bf16, tag="wp")

            qT_sb = [None, None]
            kT_sb = [None, None]
            v_sb = [None, None]
            yT_sb = [None, None]

            def make_acts(b):
                qT_sb[b] = acts.tile([128, hpc, Tp], bf16, tag="qT",
                                     name=f"qT{b}")
                kT_sb[b] = acts.tile([128, hpc, Tp], bf16, tag="kT",
                                     name=f"kT{b}")
                v_sb[b] = acts.tile([128, n_kb, jc], bf16, tag="v",
                                    name=f"v{b}")
                yT_sb[b] = acts.tile([128, hpc, Tp], bf16, tag="yT",
                                     name=f"yT{b}")

            # proj restructured around 512-token tiles (rt5), one OUTPUT
            # GROUP at a time: q-h0, q-h1, k-h0, k-h1 run 16 ck-chunks of
            # 512-free matmuls into a single PSUM bank (LDW 97ns fully
            # hides under 213ns matmuls), then v in two half-tile groups
            # (256-free, layout-forced). Unit = 2 ck steps of one group.
            RT5 = 2 * RP                      # 512-token proj tile
            n_rt5 = Tp // RT5
            GRP = [("q", 0), ("q", 1), ("k", 0), ("k", 1),
                   ("v", 0), ("v", 1)]
            proj_state = {}
            proj_xtiles = {}

            def _get_slab(b, rt5):
                slab = proj_xtiles.get((b, rt5))
                if slab is not None:
                    return slab
                t0 = b * Tp + rt5 * RT5
                slab = xpool.tile([128, n_ck, RT5], bf16, tag="xt",
                                  name=f"xt{b}_{rt5}")
                # two half-slab DMAs (quarters for the very first slab so
                # the first matmuls start earlier)
                nparts = 4 if (b == 0 and rt5 == 0) else 2
                npc = n_ck // nparts
                for pi in range(nparts):
                    c0 = pi * npc * 128
                    nc.gpsimd.dma_start(
                        slab[:, pi * npc:(pi + 1) * npc, :],
                        xT[c0:c0 + npc * 128, t0:t0 + RT5].rearrange(
                            "(c p) t -> p c t", p=128))
                proj_xtiles[(b, rt5)] = slab
                return slab

            def proj_unit(b, rt5, gi, ck2):
                kind, h = GRP[gi]
                slab = _get_slab(b, rt5)
                key = (b, rt5, gi)
                g_ps = proj_state.get(key)
                if g_ps is None:
                    g_ps = proj_state[key] = pp.tile(
                        [128, RT5], f32, tag="pp", name=f"g{b}_{rt5}_{gi}")
                tsl = slice(rt5 * RT5, (rt5 + 1) * RT5)
                for ck in (2 * ck2, 2 * ck2 + 1):
                    first = ck == 0
                    last = ck == n_ck - 1
                    if kind == "q" or kind == "k":
                        w = wq_sb[ck] if kind == "q" else wk_sb[ck]
                        nc.tensor.matmul(
                            g_ps, w[:, h * d:(h + 1) * d],
                            slab[:, ck, :], start=first, stop=last,
                            skip_group_check=True)
                    else:
                        # v half-tile: tokens [h*256, h*256+256), two
                        # 128-row sub-blocks, 256-free matmuls
                        for s in range(2):
                            nc.tensor.matmul(
                                g_ps[:, s * jc:(s + 1) * jc],
                                slab[:, ck, h * 256 + s * 128:
                                     h * 256 + (s + 1) * 128],
                                wv_sb[ck], start=(first and s == 0),
                                stop=(last and s == 1),
                                skip_group_check=True)
                if ck2 == n_ck // 2 - 1:
                    if kind in ("q", "k"):
                        dst = qT_sb[b] if kind == "q" else kT_sb[b]
                        t1 = rope.tile([d, RT5], f32, tag="t1")
                        nc.vector.tensor_mul(t1, g_ps, cos_sb[:, tsl])
                        t2 = rope.tile([d, RT5], f32, tag="t2")
                        nc.vector.tensor_mul(
                            t2[0:hd], g_ps[hd:d], sin_sb[0:hd, tsl])
                        nc.vector.tensor_mul(
                            t2[hd:d], g_ps[0:hd], sin_sb[hd:d, tsl])
                        nc.vector.tensor_add(dst[:, h, tsl], t1, t2)
                    else:
                        for s in range(2):
                            nc.scalar.activation(
                                v_sb[b][:, rt5 * 4 + h * 2 + s, :],
                                g_ps[:, s * jc:(s + 1) * jc], Copy)
                    del proj_state[key]

            attn_state = {}

            def attn_mm1(b, qt, kb):
                qsl = slice(qt * RT, (qt + 1) * RT)
                key = (b, qt)
                st = attn_state.get(key)
                if st is None:
                    ys = [ps_y.tile([d, RT], f32, tag="y",
                                    name=f"yps{b}_{qt}_{h}")
                          for h in range(hpc)]
                    das = [dpool.tile([128, RT], bf16, tag="dacc",
                                      name=f"dacc{b}_{qt}_{h}")
                           for h in range(hpc)]
                    st = attn_state[key] = (ys, das)
                ys, das = st
                es = []
                for h in range(hpc):
                    s_ps = ps_s.tile([128, RT], f32, tag="s",
                                     name=f"sps{b}_{qt}_{kb}_{h}")
                    nc.tensor.matmul(
                        s_ps,
                        kT_sb[b][:, h, kb * KB:(kb + 1) * KB],
                        qT_sb[b][:, h, qsl],
                        start=True, stop=True, skip_group_check=True)
                    if kb == 0:
                        e_sb = das[h]
                    else:
                        e_sb = epool.tile([128, RT], bf16, tag="e",
                                          name=f"esb{b}_{qt}_{kb}_{h}")
                    nc.scalar.activation(e_sb, s_ps, Exp, scale=scale)
                    if kb != 0:
                        nc.vector.tensor_add(das[h], das[h], e_sb)
                    es.append(e_sb)
                return es

            def attn_mm2(b, qt, kb, es):
                ys, das = attn_state[(b, qt)]
                for h in range(hpc):
                    nc.tensor.matmul(
                        ys[h],
                        v_sb[b][:, kb, h * d:(h + 1) * d],
                        es[h],
                        start=(kb == 0), stop=(kb == n_kb - 1),
                        skip_group_check=True)

            def attn_finalize(b, qt):
                qsl = slice(qt * RT, (qt + 1) * RT)
                ys, das = attn_state.pop((b, qt))
                for h in range(hpc):
                    dsum_ps = ps_fin.tile([1, RT], f32, tag="fin",
                                          name=f"dsum{b}_{qt}_{h}")
                    nc.tensor.matmul(dsum_ps, ones_bf, das[h],
                                     start=True, stop=True,
                                     skip_group_check=True)
                    recip_sb = small.tile([1, RT], f32, tag="recip",
                                          name=f"recip{b}_{qt}_{h}")
                    nc.vector.reciprocal_approx_fast(
                        out=recip_sb, in_=dsum_ps)
                    bc_sb = small.tile([128, RT], f32, tag="bc_sb",
                                       name=f"bcsb{b}_{qt}_{h}")
                    nc.gpsimd.partition_broadcast(
                        out_ap=bc_sb, in_ap=recip_sb)
                    nc.vector.tensor_mul(yT_sb[b][:, h, qsl], ys[h], bc_sb)

            out_state = {}
            o_copy_ctr = [0]

            def outproj_unit(b, rb, ot):
                key = (b, rb)
                o_sb = out_state.get(key)
                if o_sb is None:
                    o_sb = out_state[key] = opool.tile(
                        [128, Cp], bf16, tag="o", name=f"osb{b}_{rb}")
                o_ps = pp.tile([128, RT], f32, tag="pp",
                               name=f"ops{b}_{rb}_{ot}")
                for h in range(hpc):
                    nc.tensor.matmul(
                        o_ps,
                        yT_sb[b][:, h, rb * 128:(rb + 1) * 128],
                        wp_sb[:, h, ot * RT:(ot + 1) * RT],
                        start=(h == 0), stop=(h == hpc - 1),
                        skip_group_check=True)
                o_copy_ctr[0] += 1
                dst = o_sb[:, ot * RT:(ot + 1) * RT]
                if o_copy_ctr[0] % 4 == 0:
                    nc.scalar.activation(dst, o_ps, Copy)
                else:
                    nc.vector.tensor_copy(out=dst, in_=o_ps)
                if ot % 2 == 1:
                    # DMA out per half-row-block so the tail drains earlier
                    c0 = (ot - 1) * RT
                    nc.sync.dma_start(
                        out[b * Tp + rb * 128:b * Tp + (rb + 1) * 128,
                            c0:c0 + 2 * RT],
                        o_sb[:, c0:c0 + 2 * RT])
                    if ot == n_ot - 1:
                        del out_state[key]

            # phase 0: proj b0 with attn(b0, qt0) steps interleaved as soon
            # as their key blocks exist - these need no fresh DMA data, so
            # they fill the DMA-paced gaps of the cold-start window.
            PHASE_MARKS.append(("proj0", nc.next_id()))
            make_acts(0)
            for rt5 in range(n_rt5):
                for gi in range(len(GRP)):
                    for ck2 in range(n_ck // 2):
                        proj_unit(0, rt5, gi, ck2)
                    # attn(b0, qt0) steps lagged one rt5 behind, emitted
                    # between proj groups so their rope inputs are long
                    # done and never block the PE queue
                    if rt5 >= 1 and gi in (2, 4):
                        k0 = 4 * (rt5 - 1) + (0 if gi == 2 else 2)
                        for kb in (k0, k0 + 1):
                            es = attn_mm1(0, 0, kb)
                            attn_mm2(0, 0, kb, es)
            for kb in range(4 * (n_rt5 - 1), n_kb):
                es = attn_mm1(0, 0, kb)
                attn_mm2(0, 0, kb, es)
            attn_finalize(0, 0)

            # phase 1: attn b0 qt1..3 with proj b1 units as PE filler
            PHASE_MARKS.append(("attn0", nc.next_id()))
            make_acts(1)
            nc.scalar.dma_start(
                wp_sb, wp.rearrange("(h p) o -> p h o", p=128))
            filler1 = [(1, rt5, gi, ck2) for rt5 in range(n_rt5)
                       for gi in range(len(GRP))
                       for ck2 in range(n_ck // 2)]
            fi = 0
            n_steps = (n_qt - 1) * n_kb
            step = 0
            for qt in range(1, n_qt):
                for kb in range(n_kb):
                    es = attn_mm1(0, qt, kb)
                    step += 1
                    tgt = step * len(filler1) // n_steps
                    while fi < tgt:
                        proj_unit(*filler1[fi])
                        fi += 1
                    attn_mm2(0, qt, kb, es)
                attn_finalize(0, qt)
            while fi < len(filler1):
                proj_unit(*filler1[fi])
                fi += 1

            PHASE_MARKS.append(("attn1", nc.next_id()))
            from collections import deque
            ounits = deque((0, rb, ot) for rb in range(n_rb)
                           for ot in range(n_ot))
            for qt in range(n_qt):
                for kb in range(n_kb):
                    es = attn_mm1(1, qt, kb)
                    for _ in range(2):
                        if ounits:
                            outproj_unit(*ounits.popleft())
                    attn_mm2(1, qt, kb, es)
                attn_finalize(1, qt)
                for rb in range(qt * n_rb // n_qt,
                                (qt + 1) * n_rb // n_qt):
                    for ot in range(n_ot):
                        ounits.append((1, rb, ot))

            PHASE_MARKS.append(("tail", nc.next_id()))
            while ounits:
                outproj_unit(*ounits.popleft())

    PHASE_MARKS.append(("end", nc.next_id()))
    nc.compile()
    return nc


def _prep_in_maps(x, cos, sin, W_qkv, W_proj, n_cores, hpc, d):
    Bp, Tp, Cp = x.shape
    jc = hpc * d
    import ml_dtypes
    xTa = np.ascontiguousarray(x.reshape(Bp * Tp, Cp).T).astype(ml_dtypes.bfloat16)
    cosT = np.ascontiguousarray(cos.T)
    sinT = np.ascontiguousarray(sin.T).copy()
    sinT[: d // 2] *= -1.0
    in_maps = []
    for c in range(n_cores):
        j0, j1 = c * jc, (c + 1) * jc
        in_maps.append({
            "xT": xTa,
            "wqkv": np.ascontiguousarray(np.concatenate(
                [W_qkv[:, j0:j1], W_qkv[:, Cp + j0:Cp + j1],
                 W_qkv[:, 2 * Cp + j0:2 * Cp + j1]], axis=1,
            )).astype(ml_dtypes.bfloat16),
            "wp": np.ascontiguousarray(W_proj[j0:j1, :]).astype(ml_dtypes.bfloat16),
            "ones": np.ones((128, 1), dtype=np.float32),
            "cosT": cosT,
            "sinT": sinT,
        })
    return in_maps


def _install_ntff_hook():
    import sys
    import types
    try:
        from antenv.axon_hooks import get_axon_ntff_profile_hook
        if get_axon_ntff_profile_hook() is not None:
            return
    except ImportError:
        pass
    try:
        sys.path.insert(0, "/root/.axon_site")
        from trn_agent_boot.trn_boot import _ntff_profile_via_ctypes

        hook = _ntff_profile_via_ctypes("/opt/axon/libaxon_pjrt.so")
        if hook is None:
            return
        mod = types.ModuleType("antenv.axon_hooks")
        mod.get_axon_ntff_profile_hook = lambda: hook
        mod.set_axon_ntff_profile_hook = lambda h: None
        import antenv
        antenv.axon_hooks = mod
        sys.modules["antenv.axon_hooks"] = mod
    except Exception:
        pass


def _run(x, cos, sin, W_qkv, W_proj, trace=False):
    from concourse.bass_utils import run_bass_kernel_spmd

    if trace:
        _install_ntff_hook()

    x = np.ascontiguousarray(x, dtype=np.float32)
    cos = np.ascontiguousarray(cos, dtype=np.float32)
    sin = np.ascontiguousarray(sin, dtype=np.float32)
    W_qkv = np.ascontiguousarray(W_qkv, dtype=np.float32)
    W_proj = np.ascontiguousarray(W_proj, dtype=np.float32)

    Bp, Tp, Cp = x.shape
    nc = _build(Bp, Tp, Cp, HPC, D)
    in_maps = _prep_in_maps(x, cos, sin, W_qkv, W_proj, N_CORES, HPC, D)
    res = run_bass_kernel_spmd(nc, in_maps, core_ids=list(range(N_CORES)),
                               trace=trace)
    acc = np.zeros((Bp * Tp, Cp), dtype=np.float32)
    for i in range(N_CORES):
        acc += np.asarray(res.results[i]["out"], dtype=np.float32)
    return acc.reshape(Bp, Tp, Cp), res


def kernel(x, cos, sin, W_qkv, W_proj):
    out, _ = _run(x, cos, sin, W_qkv, W_proj, trace=False)
    return out


# revision 33
# speedup vs baseline: 1.1938x; 1.1938x over previous
"""Trainium2 Bass kernel: multi-head attention (B=2, T=2048, C=2048, H=16, D=128).

v2 reconstruction (A/B calibration vs v4 under current HW power state).

Sharding: tensor-parallel over heads. 8 cores x 2 heads each.
Per-core dataflow and scheduling: see kernel_v4.py docstring.
"""

import math

import numpy as np

N_CORES = 8
B, T, C = 2, 2048, 2048
N_HEAD, D = 16, 128
HPC = N_HEAD // N_CORES          # heads per core
JC = HPC * D                     # per-core slice width of qkv/proj dims

RT = 512                         # query tile (moving free dim) in attention
RP = 256                         # token tile for qkv projection
KB = 128                         # key block (contraction tile) in attention

PHASE_MARKS = []


def _build(Bp, Tp, Cp, hpc, d):
    PHASE_MARKS.clear()
    import concourse.bacc as bacc
    import concourse.tile as tile
    from concourse import mybir

    f32 = mybir.dt.float32
    bf16 = mybir.dt.bfloat16
    Exp = mybir.ActivationFunctionType.Exp
    Copy = mybir.ActivationFunctionType.Copy

    jc = hpc * d
    BT = Bp * Tp
    n_ck = Cp // 128
    n_rt = Tp // RP
    n_kb = Tp // KB
    n_qt = Tp // RT
    n_rb = Tp // 128
    n_ot = Cp // RT
    hd = d // 2
    scale = 1.0 / math.sqrt(d)

    nc = bacc.Bacc("TRN2", target_bir_lowering=False, debug=False)

    xT = nc.declare_dram_parameter("xT", [Cp, BT], bf16, isOutput=False)
    wqkv = nc.declare_dram_parameter("wqkv", [Cp, 3 * jc], bf16,
                                     isOutput=False)
    wp = nc.declare_dram_parameter("wp", [jc, Cp], bf16, isOutput=False)
    ones_d = nc.declare_dram_parameter("ones", [128, 1], f32, isOutput=False)
    cosT = nc.declare_dram_parameter("cosT", [d, Tp], f32, isOutput=False)
    sinT = nc.declare_dram_parameter("sinT", [d, Tp], f32, isOutput=False)
    out = nc.declare_dram_parameter("out", [BT, Cp], bf16, isOutput=True)

    with tile.TileContext(nc) as tc:
        with (
            nc.allow_low_precision(reason="bf16 matmuls, fp32 PSUM accum"),
            tc.tile_pool(name="wpool", bufs=1) as wpool,
            tc.tile_pool(name="acts", bufs=2) as acts,
            tc.tile_pool(name="xpool", bufs=2) as xpool,
            tc.tile_pool(name="rope", bufs=3) as rope,
            tc.tile_pool(name="epool", bufs=8) as epool,
            tc.tile_pool(name="dpool", bufs=4) as dpool,
            tc.tile_pool(name="small", bufs=3) as small,
            tc.tile_pool(name="opool", bufs=4) as opool,
            tc.tile_pool(name="pp", bufs=3, space="PSUM") as pp,
            tc.tile_pool(name="ps_s", bufs=2, space="PSUM") as ps_s,
            tc.tile_pool(name="ps_y", bufs=2, space="PSUM") as ps_y,
            tc.tile_pool(name="ps_fin", bufs=1, space="PSUM") as ps_fin,
        ):
            wq_sb, wk_sb, wv_sb = [], [], []
            for ck in range(n_ck):
                t = wpool.tile([128, 3 * jc], bf16, tag=f"w{ck}",
                               name=f"w{ck}")
                eng = nc.sync if ck % 2 == 0 else nc.scalar
                eng.dma_start(t, wqkv[ck * 128:(ck + 1) * 128, :])
                wq_sb.append(t[:, 0:jc])
                wk_sb.append(t[:, jc:2 * jc])
                wv_sb.append(t[:, 2 * jc:3 * jc])
            cos_sb = wpool.tile([d, Tp], f32, tag="cos")
            sin_sb = wpool.tile([d, Tp], f32, tag="sin")
            nc.scalar.dma_start(cos_sb, cosT[:])
            nc.scalar.dma_start(sin_sb, sinT[:])
            ones_sb = wpool.tile([128, 1], f32, tag="ones")
            nc.sync.dma_start(ones_sb, ones_d[:])
            ones_bf = wpool.tile([128, 1], bf16, tag="ones_bf")
            nc.vector.tensor_copy(out=ones_bf, in_=ones_sb)
            wp_sb = wpool.tile([128, hpc, Cp], bf16, tag="wp")

            qT_sb = [None, None]
            kT_sb = [None, None]
            v_sb = [None, None]
            yT_sb = [None, None]

            def make_acts(b):
                qT_sb[b] = acts.tile([128, hpc, Tp], bf16, tag="qT",
                                     name=f"qT{b}")
                kT_sb[b] = acts.tile([128, hpc, Tp], bf16, tag="kT",
                                     name=f"kT{b}")
                v_sb[b] = acts.tile([128, n_kb, jc], bf16, tag="v",
                                    name=f"v{b}")
                yT_sb[b] = acts.tile([128, hpc, Tp], bf16, tag="yT",
                                     name=f"yT{b}")

            # proj restructured around 512-token tiles (rt5), one OUTPUT
            # GROUP at a time: q-h0, q-h1, k-h0, k-h1 run 16 ck-chunks of
            # 512-free matmuls into a single PSUM bank (LDW 97ns fully
            # hides under 213ns matmuls), then v in two half-tile groups
            # (256-free, layout-forced). Unit = 2 ck steps of one group.
            RT5 = 2 * RP                      # 512-token proj tile
            n_rt5 = Tp // RT5
            GRP = [("q", 0), ("q", 1), ("k", 0), ("k", 1),
                   ("v", 0), ("v", 1)]
            proj_state = {}
            proj_xtiles = {}

            def _get_slab(b, rt5):
                slab = proj_xtiles.get((b, rt5))
                if slab is not None:
                    return slab
                t0 = b * Tp + rt5 * RT5
                slab = xpool.tile([128, n_ck, RT5], bf16, tag="xt",
                                  name=f"xt{b}_{rt5}")
                # two half-slab DMAs (eighths for the very first slab so
                # the first matmuls start earlier)
                nparts = 8 if (b == 0 and rt5 == 0) else 2
                npc = n_ck // nparts
                for pi in range(nparts):
                    c0 = pi * npc * 128
                    nc.gpsimd.dma_start(
                        slab[:, pi * npc:(pi + 1) * npc, :],
                        xT[c0:c0 + npc * 128, t0:t0 + RT5].rearrange(
                            "(c p) t -> p c t", p=128))
                proj_xtiles[(b, rt5)] = slab
                return slab

            def proj_unit(b, rt5, gi, ck2):
                kind, h = GRP[gi]
                slab = _get_slab(b, rt5)
                key = (b, rt5, gi)
                g_ps = proj_state.get(key)
                if g_ps is None:
                    g_ps = proj_state[key] = pp.tile(
                        [128, RT5], f32, tag="pp", name=f"g{b}_{rt5}_{gi}")
                tsl = slice(rt5 * RT5, (rt5 + 1) * RT5)
                for ck in (2 * ck2, 2 * ck2 + 1):
                    first = ck == 0
                    last = ck == n_ck - 1
                    if kind == "q" or kind == "k":
                        w = wq_sb[ck] if kind == "q" else wk_sb[ck]
                        nc.tensor.matmul(
                            g_ps, w[:, h * d:(h + 1) * d],
                            slab[:, ck, :], start=first, stop=last,
                            skip_group_check=True)
                    else:
                        # v half-tile: tokens [h*256, h*256+256), two
                        # 128-row sub-blocks, 256-free matmuls
                        for s in range(2):
                            nc.tensor.matmul(
                                g_ps[:, s * jc:(s + 1) * jc],
                                slab[:, ck, h * 256 + s * 128:
                                     h * 256 + (s + 1) * 128],
                                wv_sb[ck], start=(first and s == 0),
                                stop=(last and s == 1),
                                skip_group_check=True)
                if ck2 == n_ck // 2 - 1:
                    if kind in ("q", "k"):
                        dst = qT_sb[b] if kind == "q" else kT_sb[b]
                        t1 = rope.tile([d, RT5], f32, tag="t1")
                        nc.vector.tensor_mul(t1, g_ps, cos_sb[:, tsl])
                        t2 = rope.tile([d, RT5], f32, tag="t2")
                        nc.vector.tensor_mul(
                            t2[0:hd], g_ps[hd:d], sin_sb[0:hd, tsl])
                        nc.vector.tensor_mul(
                            t2[hd:d], g_ps[0:hd], sin_sb[hd:d, tsl])
                        nc.vector.tensor_add(dst[:, h, tsl], t1, t2)
                    else:
                        for s in range(2):
                            nc.scalar.activation(
                                v_sb[b][:, rt5 * 4 + h * 2 + s, :],
                                g_ps[:, s * jc:(s + 1) * jc], Copy)
                    del proj_state[key]

            attn_state = {}

            def attn_mm1(b, qt, kb):
                qsl = slice(qt * RT, (qt + 1) * RT)
                key = (b, qt)
                st = attn_state.get(key)
                if st is None:
                    ys = [ps_y.tile([d, RT], f32, tag="y",
                                    name=f"yps{b}_{qt}_{h}")
                          for h in range(hpc)]
                    das = [dpool.tile([128, RT], bf16, tag="dacc",
                                      name=f"dacc{b}_{qt}_{h}")
                           for h in range(hpc)]
                    st = attn_state[key] = (ys, das)
                ys, das = st
                es = []
                for h in range(hpc):
                    s_ps = ps_s.tile([128, RT], f32, tag="s",
                                     name=f"sps{b}_{qt}_{kb}_{h}")
                    nc.tensor.matmul(
                        s_ps,
                        kT_sb[b][:, h, kb * KB:(kb + 1) * KB],
                        qT_sb[b][:, h, qsl],
                        start=True, stop=True, skip_group_check=True)
                    if kb == 0:
                        e_sb = das[h]
                    else:
                        e_sb = epool.tile([128, RT], bf16, tag="e",
                                          name=f"esb{b}_{qt}_{kb}_{h}")
                    nc.scalar.activation(e_sb, s_ps, Exp, scale=scale)
                    if kb != 0:
                        nc.vector.tensor_add(das[h], das[h], e_sb)
                    es.append(e_sb)
                return es

            def attn_mm2(b, qt, kb, es):
                ys, das = attn_state[(b, qt)]
                for h in range(hpc):
                    nc.tensor.matmul(
                        ys[h],
                        v_sb[b][:, kb, h * d:(h + 1) * d],
                        es[h],
                        start=(kb == 0), stop=(kb == n_kb - 1),
                        skip_group_check=True)

            def attn_finalize(b, qt):
                qsl = slice(qt * RT, (qt + 1) * RT)
                ys, das = attn_state.pop((b, qt))
                for h in range(hpc):
                    dsum_ps = ps_fin.tile([1, RT], f32, tag="fin",
                                          name=f"dsum{b}_{qt}_{h}")
                    nc.tensor.matmul(dsum_ps, ones_bf, das[h],
                                     start=True, stop=True,
                                     skip_group_check=True)
                    recip_sb = small.tile([1, RT], f32, tag="recip",
                                          name=f"recip{b}_{qt}_{h}")
                    nc.vector.reciprocal_approx_fast(
                        out=recip_sb, in_=dsum_ps)
                    bc_sb = small.tile([128, RT], f32, tag="bc_sb",
                                       name=f"bcsb{b}_{qt}_{h}")
                    nc.gpsimd.partition_broadcast(
                        out_ap=bc_sb, in_ap=recip_sb)
                    nc.vector.tensor_mul(yT_sb[b][:, h, qsl], ys[h], bc_sb)

            out_state = {}
            o_copy_ctr = [0]

            def outproj_unit(b, rb, ot):
                key = (b, rb)
                o_sb = out_state.get(key)
                if o_sb is None:
                    o_sb = out_state[key] = opool.tile(
                        [128, Cp], bf16, tag="o", name=f"osb{b}_{rb}")
                o_ps = pp.tile([128, RT], f32, tag="pp",
                               name=f"ops{b}_{rb}_{ot}")
                for h in range(hpc):
                    nc.tensor.matmul(
                        o_ps,
                        yT_sb[b][:, h, rb * 128:(rb + 1) * 128],
                        wp_sb[:, h, ot * RT:(ot + 1) * RT],
                        start=(h == 0), stop=(h == hpc - 1),
                        skip_group_check=True)
                o_copy_ctr[0] += 1
                dst = o_sb[:, ot * RT:(ot + 1) * RT]
                if o_copy_ctr[0] % 4 == 0:
                    nc.scalar.activation(dst, o_ps, Copy)
                else:
                    nc.vector.tensor_copy(out=dst, in_=o_ps)
                if ot % 2 == 1:
                    # DMA out per half-row-block so the tail drains earlier
                    c0 = (ot - 1) * RT
                    nc.sync.dma_start(
                        out[b * Tp + rb * 128:b * Tp + (rb + 1) * 128,
                            c0:c0 + 2 * RT],
                        o_sb[:, c0:c0 + 2 * RT])
                    if ot == n_ot - 1:
                        del out_state[key]

            # phase 0: proj b0 with attn(b0, qt0) steps interleaved as soon
            # as their key blocks exist - these need no fresh DMA data, so
            # they fill the DMA-paced gaps of the cold-start window.
            PHASE_MARKS.append(("proj0", nc.next_id()))
            make_acts(0)
            for rt5 in range(n_rt5):
                for gi in range(len(GRP)):
                    for ck2 in range(n_ck // 2):
                        proj_unit(0, rt5, gi, ck2)
                    # attn(b0, qt0) steps lagged one rt5 behind, emitted
                    # between proj groups so their rope inputs are long
                    # done and never block the PE queue
                    if rt5 >= 1 and gi in (2, 4):
                        k0 = 4 * (rt5 - 1) + (0 if gi == 2 else 2)
                        for kb in (k0, k0 + 1):
                            es = attn_mm1(0, 0, kb)
                            attn_mm2(0, 0, kb, es)
            # phase 1: rest of attn b0 (qt0 tail + qt1..3) with proj b1
            # units as PE filler so no step runs dry
            PHASE_MARKS.append(("attn0", nc.next_id()))
            make_acts(1)
            nc.scalar.dma_start(
                wp_sb, wp.rearrange("(h p) o -> p h o", p=128))
            filler1 = [(1, rt5, gi, ck2) for rt5 in range(n_rt5)
                       for gi in range(len(GRP))
                       for ck2 in range(n_ck // 2)]
            steps1 = [(0, kb) for kb in range(4 * (n_rt5 - 1), n_kb)]
            steps1 += [(qt, kb) for qt in range(1, n_qt)
                       for kb in range(n_kb)]
            fi = 0
            for step, (qt, kb) in enumerate(steps1):
                es = attn_mm1(0, qt, kb)
                tgt = (step + 1) * len(filler1) // len(steps1)
                while fi < tgt:
                    proj_unit(*filler1[fi])
                    fi += 1
                attn_mm2(0, qt, kb, es)
                if kb == n_kb - 1:
                    attn_finalize(0, qt)
            while fi < len(filler1):
                proj_unit(*filler1[fi])
                fi += 1

            PHASE_MARKS.append(("attn1", nc.next_id()))
            from collections import deque
            ounits = deque((0, rb, ot) for rb in range(n_rb)
                           for ot in range(n_ot))
            for qt in range(n_qt):
                for kb in range(n_kb):
                    es = attn_mm1(1, qt, kb)
                    for _ in range(2):
                        if ounits:
                            outproj_unit(*ounits.popleft())
                    attn_mm2(1, qt, kb, es)
                attn_finalize(1, qt)
                for rb in range(qt * n_rb // n_qt,
                                (qt + 1) * n_rb // n_qt):
                    for ot in range(n_ot):
                        ounits.append((1, rb, ot))

            PHASE_MARKS.append(("tail", nc.next_id()))
            while ounits:
                outproj_unit(*ounits.popleft())

    PHASE_MARKS.append(("end", nc.next_id()))
    nc.compile()
    return nc


def _prep_in_maps(x, cos, sin, W_qkv, W_proj, n_cores, hpc, d):
    Bp, Tp, Cp = x.shape
    jc = hpc * d
    import ml_dtypes
    xTa = np.ascontiguousarray(x.reshape(Bp * Tp, Cp).T).astype(ml_dtypes.bfloat16)
    cosT = np.ascontiguousarray(cos.T)
    sinT = np.ascontiguousarray(sin.T).copy()
    sinT[: d // 2] *= -1.0
    in_maps = []
    for c in range(n_cores):
        j0, j1 = c * jc, (c + 1) * jc
        in_maps.append({
            "xT": xTa,
            "wqkv": np.ascontiguousarray(np.concatenate(
                [W_qkv[:, j0:j1], W_qkv[:, Cp + j0:Cp + j1],
                 W_qkv[:, 2 * Cp + j0:2 * Cp + j1]], axis=1,
            )).astype(ml_dtypes.bfloat16),
            "wp": np.ascontiguousarray(W_proj[j0:j1, :]).astype(ml_dtypes.bfloat16),
            "ones": np.ones((128, 1), dtype=np.float32),
            "cosT": cosT,
            "sinT": sinT,
        })
    return in_maps


def _install_ntff_hook():
    import sys
    import types
    try:
        from antenv.axon_hooks import get_axon_ntff_profile_hook
        if get_axon_ntff_profile_hook() is not None:
            return
    except ImportError:
        pass
    try:
        sys.path.insert(0, "/root/.axon_site")
        from trn_agent_boot.trn_boot import _ntff_profile_via_ctypes

        hook = _ntff_profile_via_ctypes("/opt/axon/libaxon_pjrt.so")
        if hook is None:
            return
        mod = types.ModuleType("antenv.axon_hooks")
        mod.get_axon_ntff_profile_hook = lambda: hook
        mod.set_axon_ntff_profile_hook = lambda h: None
        import antenv
        antenv.axon_hooks = mod
        sys.modules["antenv.axon_hooks"] = mod
    except Exception:
        pass


def _run(x, cos, sin, W_qkv, W_proj, trace=False):
    from concourse.bass_utils import run_bass_kernel_spmd

    if trace:
        _install_ntff_hook()

    x = np.ascontiguousarray(x, dtype=np.float32)
    cos = np.ascontiguousarray(cos, dtype=np.float32)
    sin = np.ascontiguousarray(sin, dtype=np.float32)
    W_qkv = np.ascontiguousarray(W_qkv, dtype=np.float32)
    W_proj = np.ascontiguousarray(W_proj, dtype=np.float32)

    Bp, Tp, Cp = x.shape
    nc = _build(Bp, Tp, Cp, HPC, D)
    in_maps = _prep_in_maps(x, cos, sin, W_qkv, W_proj, N_CORES, HPC, D)
    res = run_bass_kernel_spmd(nc, in_maps, core_ids=list(range(N_CORES)),
                               trace=trace)
    acc = np.zeros((Bp * Tp, Cp), dtype=np.float32)
    for i in range(N_CORES):
        acc += np.asarray(res.results[i]["out"], dtype=np.float32)
    return acc.reshape(Bp, Tp, Cp), res


def kernel(x, cos, sin, W_qkv, W_proj):
    out, _ = _run(x, cos, sin, W_qkv, W_proj, trace=False)
    return out
